# revision 1
# baseline (speedup 1.0000x reference)
"""Trainium2 Bass kernel for the CNN-MAD per-class DTW transport cost.

Math (reference):
  mat_cost[n, j] = C1[n] + C2[c_n, j] - 2*C3[n, j],  c_n = classes[n]
    C1[n]    = sum_t rowsum[c_n, t] * ||X[n,t,:]||^2
    C2[c, j] = sum_p colsum[c, p] * ||Y[j,p,:]||^2
    C3[n, j] = sum_{p,d} (sum_t pi[c_n,t,p] X[n,t,d]) * Y[j,p,d]

Sharding: one class per core (C == n_cores == 8). Host groups samples by
class (pure gather / re-layout, no arithmetic), each core computes the
[NY, CAP] transposed block for its class against the full Y, and the host
scatters rows back into the [N, NY] output.

Device per core (class k), all f32:
  - pi    [T, TP]      : class-k DTW matrix
  - xt2   [T, D*CAP]   : X.T re-layout, xt2[t, d*CAP+n] = Xg[n, t, d]
  - yt    [D*TP, NY]   : Y.T re-layout, yt[d*TP+p, j]  = Y[j, p, d]
  rowsum via DVE reduce; colsum via matmul with ones; C1 via matmul of
  rowsum over squared xt2; C2 via ACT squares and a fused DVE
  scale-accumulate chain plus a ones-contraction matmul; XW = pi.T @ X
  per d; final transposed result outT[j, n] accumulated kc-outer across
  8 concurrently-open PSUM banks as sum_kc yt_kc.T @ (-2*XW)_kc with a
  trailing [C2;1].T [1;C1] augmentation matmul per 128-row block.
"""

import sys

sys.path.insert(0, "/opt/trn_rl_repo")

import numpy as np

N, NY, T, TP, D, C = 1024, 1024, 256, 256, 8, 8
NCORES = 8

_cache = {}


def _build(cap):
    import concourse.bacc as bacc
    import concourse.mybir as mybir
    import concourse.tile as tile

    dt = mybir.dt.float32
    nc = bacc.Bacc("TRN2", target_bir_lowering=False, debug=False, num_devices=NCORES)

    pi_d = nc.dram_tensor("pi", [T, TP], dt, kind="ExternalInput")
    xt2_d = nc.dram_tensor("xt2", [T, D * cap], dt, kind="ExternalInput")
    yt_d = nc.dram_tensor("yt", [D * TP, NY], dt, kind="ExternalInput")
    out_d = nc.dram_tensor("outT", [NY, cap], dt, kind="ExternalOutput")

    KC = D * TP // 128  # 16 yt chunks of 128 contraction rows
    XF = D * cap        # xt2 free size
    JT = NY // 128      # 8 output partition tiles (transposed layout)
    # XW psum segments, aligned to d-blocks and <= 512 f32 (one PSUM bank)
    nd_max = max(1, 512 // cap)
    DSEG = [(i, min(nd_max, D - i)) for i in range(0, D, nd_max)]

    with tile.TileContext(nc) as tc:
        with (
            tc.tile_pool(name="const", bufs=1) as pconst,
            tc.tile_pool(name="xin", bufs=1) as px,
            tc.tile_pool(name="ytp", bufs=1) as pyt,
            tc.tile_pool(name="ysqw", bufs=6) as pysq,
            tc.tile_pool(name="xwt", bufs=1) as pxwt,
            tc.tile_pool(name="osb", bufs=8) as posb,
            tc.tile_pool(name="ps", bufs=8, space="PSUM") as psp,
        ):
            # ---- input DMAs: pi/xt2 on gpsimd SWDGE, yt chunks on SP HWDGE ----
            pi_sb = []
            for tch in range(2):
                p = pconst.tile([128, TP], dt, tag=f"pi{tch}")
                nc.sync.dma_start(p[:], pi_d[tch * 128 : (tch + 1) * 128, :])
                pi_sb.append(p)
            xt2 = []
            for tch in range(2):
                xt = px.tile([128, XF], dt, tag=f"xt2_{tch}")
                for d0, nd in DSEG:
                    nc.sync.dma_start(
                        xt[:, d0 * cap : (d0 + nd) * cap],
                        xt2_d[
                            tch * 128 : (tch + 1) * 128,
                            d0 * cap : (d0 + nd) * cap,
                        ],
                    )
                xt2.append(xt)
            yt = pyt.tile([128, KC * NY], dt, tag="yt")
            for kc in range(KC):
                nc.sync.dma_start(
                    yt[:, kc * NY : (kc + 1) * NY],
                    yt_d[kc * 128 : (kc + 1) * 128, :],
                )

            # ---- rowsum (DVE free-dim reduce), ones, colsum (PE) ----
            rowsum = []
            for tch in range(2):
                r = pconst.tile([128, 1], dt, tag=f"rowsum{tch}")
                nc.vector.reduce_sum(r[:], pi_sb[tch][:], axis=mybir.AxisListType.X)
                rowsum.append(r)
            ones = pconst.tile([128, 1], dt, tag="ones")
            nc.vector.memset(ones[:], 1.0)

            cs_ps = psp.tile([128, 2], dt, tag="ps8", name="cs_ps")
            for pc in range(2):
                for tch in range(2):
                    nc.tensor.matmul(
                        cs_ps[:, pc : pc + 1],
                        pi_sb[tch][:, pc * 128 : (pc + 1) * 128],
                        ones[:],
                        start=(tch == 0),
                        stop=(tch == 1),
                    )
            colsum_sb = pconst.tile([128, 2], dt, tag="colsum_sb")
            nc.vector.tensor_copy(colsum_sb[:], cs_ps[:])
            colsum = [colsum_sb[:, 0:1], colsum_sb[:, 1:2]]

            # ---- XW: per p-half, out [128p, (d,n)] = pi_half.T @ xt2 ----
            # xwt viewed [128, d, pc, n]: chunk kc = d*2+pc of (-2*XW).T
            xwt = pxwt.tile([128, KC * cap], dt, tag="xwt")
            xwt_v = xwt.rearrange("l (d pc n) -> l d pc n", pc=2, n=cap)
            xw_ps = {
                (pc, d0): psp.tile(
                    [128, nd * cap], dt, tag="ps8", name=f"xwps{pc}_{d0}"
                )
                for pc in range(2)
                for d0, nd in DSEG
            }
            # tch-outer so PE starts on xt2[0] before xt2[1] lands
            for tch in range(2):
                for pc in range(2):
                    for d0, nd in DSEG:
                        nc.tensor.matmul(
                            xw_ps[(pc, d0)][:],
                            pi_sb[tch][:, pc * 128 : (pc + 1) * 128],
                            xt2[tch][:, d0 * cap : (d0 + nd) * cap],
                            start=(tch == 0),
                            stop=(tch == 1),
                        )
            for pc in range(2):
                for d0, nd in DSEG:
                    # ACT evac with -2 scale into strided chunk layout
                    nc.scalar.mul(
                        xwt_v[:, d0 : d0 + nd, pc, :],
                        xw_ps[(pc, d0)].rearrange("l (d n) -> l d n", n=cap),
                        -2.0,
                    )

            # ---- xt2 squares + C1 row ----
            xt2sq = []
            for tch in range(2):
                xsq = px.tile([128, XF], dt, tag=f"xt2sq_{tch}")
                nc.scalar.square(xsq[:], xt2[tch][:])
                xt2sq.append(xsq)

            # ---- C2 partial sums: squares split ACT/Pool + fused DVE
            # scale-accumulate (after the XW evacs so ACT frees xwt first) ----
            ssum = pconst.tile([128, NY], dt, tag="ssum")
            for kc in range(KC):
                ysq = pysq.tile([128, NY], dt, tag="ysq")
                ysrc = yt[:, kc * NY : (kc + 1) * NY]
                if kc < 7:
                    # Pool is free early; ACT is busy with XW evacs at first
                    nc.gpsimd.tensor_mul(ysq[:], ysrc, ysrc)
                else:
                    nc.scalar.square(ysq[:], ysrc)
                if kc == 0:
                    nc.vector.tensor_scalar_mul(ssum[:], ysq[:], colsum[0][:])
                else:
                    nc.vector.scalar_tensor_tensor(
                        ssum[:],
                        ysq[:],
                        colsum[kc % 2][:],
                        ssum[:],
                        op0=mybir.AluOpType.mult,
                        op1=mybir.AluOpType.add,
                    )
            # d-reduce the squares on DVE, then one small K=128 contraction
            c1row = pconst.tile([1, cap], dt, tag="c1row")
            c1_ps = psp.tile([1, cap], dt, tag="ps8", name="c1_ps")
            xsq_dsum = []
            for tch in range(2):
                xd = px.tile([128, cap], dt, tag=f"xsq_dsum{tch}")
                nc.vector.reduce_sum(
                    xd[:],
                    xt2sq[tch].rearrange("l (d n) -> l n d", n=cap),
                    axis=mybir.AxisListType.X,
                )
                xsq_dsum.append(xd)
            for tch in range(2):
                nc.tensor.matmul(
                    c1_ps[0:1, :],
                    rowsum[tch][:],
                    xsq_dsum[tch][:],
                    start=(tch == 0),
                    stop=(tch == 1),
                )
            nc.vector.tensor_copy(c1row[0:1, :], c1_ps[0:1, :])
            # aug rhs [2, cap]: row0 = ones, row1 = C1row (SBUF->SBUF DMA;
            # compute engines cannot write at partition base 1)
            aug_r = pconst.tile([2, cap], dt, tag="aug_r")
            nc.vector.memset(aug_r[:], 1.0)
            nc.sync.dma_start(aug_r[1:2, :], c1row[0:1, :])

            # ---- C2 row: ones-contraction of ssum (own 2-bank pool, so slot
            # waits never block the C3 PE stream) ----
            aug_l = pconst.tile([2, NY], dt, tag="aug_l")
            nc.vector.memset(aug_l[:], 1.0)
            # partition-axis reduction on Pool, straight into aug_l row 0
            nc.gpsimd.reduce_sum(
                aug_l[0:1, :], ssum[:], axis=mybir.AxisListType.C
            )

            # ---- C3 transposed, kc-outer, all 8 groups open at once: three
            # jt-groups share each PSUM bank (cap*3 <= 512 f32) ----
            gsz = 512 // cap  # groups per psum tile
            ntile = -(-JT // gsz)
            pstiles = [
                psp.tile([128, min(gsz, JT - i * gsz) * cap], dt, tag="ps8",
                         name=f"psc3_{i}")
                for i in range(ntile)
            ]

            def pslice(jt):
                return pstiles[jt // gsz][:, (jt % gsz) * cap : (jt % gsz + 1) * cap]

            for kc in range(KC):
                for jt in range(JT):
                    nc.tensor.matmul(
                        pslice(jt),
                        yt[:, kc * NY + jt * 128 : kc * NY + (jt + 1) * 128],
                        xwt[:, kc * cap : (kc + 1) * cap],
                        start=(kc == 0 and jt % gsz == 0),
                        stop=False,
                        skip_group_check=True,
                    )
            # close groups bank-major: all augs of a bank, then its evacs, so
            # the same-bank PE-write/DVE-read serialization never ping-pongs
            for i in range(ntile):
                jts = range(i * gsz, min((i + 1) * gsz, JT))
                for jt in jts:
                    nc.tensor.matmul(
                        pslice(jt),
                        aug_l[:, jt * 128 : (jt + 1) * 128],
                        aug_r[:],
                        start=False,
                        stop=True,
                    )
                osb = posb.tile(
                    [128, len(jts) * cap], dt, tag=f"osb{i}", name=f"osb{i}"
                )
                for k, jt in enumerate(jts):
                    nc.vector.tensor_copy(
                        osb[:, k * cap : (k + 1) * cap], pslice(jt)
                    )
                # one DMA per bank: DRAM view [l, jt, n] pairs with SBUF
                # [l(part), jt, n]
                j0 = i * gsz
                nc.sync.dma_start(
                    out_d.rearrange("(jt l) n -> l jt n", l=128)[
                        :, j0 : j0 + len(jts), :
                    ],
                    osb.rearrange("l (jt n) -> l jt n", n=cap),
                )

    nc.compile()
    return nc


def kernel(X, Y, pi_dtw, classes):
    from concourse.bass_utils import run_bass_kernel_spmd

    X = np.ascontiguousarray(np.asarray(X, dtype=np.float32))
    Y = np.ascontiguousarray(np.asarray(Y, dtype=np.float32))
    pi_dtw = np.ascontiguousarray(np.asarray(pi_dtw, dtype=np.float32))
    classes = np.asarray(classes).astype(np.int64)

    counts = np.bincount(classes, minlength=C)
    cap = max(96, int(-(-int(counts.max()) // 8) * 8))

    if cap not in _cache:
        _cache[cap] = _build(cap)
    nc = _cache[cap]

    # host-side re-layouts (data movement only, no arithmetic)
    yt = np.ascontiguousarray(Y.transpose(2, 1, 0).reshape(D * TP, NY))
    idx = [np.nonzero(classes == c)[0] for c in range(C)]
    in_maps = []
    for c in range(C):
        xg = np.zeros((cap, T, D), dtype=np.float32)
        xg[: counts[c]] = X[idx[c]]
        xt2 = np.ascontiguousarray(xg.transpose(1, 2, 0).reshape(T, D * cap))
        in_maps.append(
            {"pi": np.ascontiguousarray(pi_dtw[c]), "xt2": xt2, "yt": yt}
        )

    res = run_bass_kernel_spmd(nc, in_maps, core_ids=list(range(NCORES)))

    out = np.empty((N, NY), dtype=np.float32)
    for c in range(C):
        out[idx[c]] = res.results[c]["outT"].T[: counts[c]]
    return out



# revision 2
# speedup vs baseline: 1.0039x; 1.0039x over previous
"""Trainium2 Bass kernel for the CNN-MAD per-class DTW transport cost.

Math (reference):
  mat_cost[n, j] = C1[n] + C2[c_n, j] - 2*C3[n, j],  c_n = classes[n]
    C1[n]    = sum_{t,d} rowsum[c_n, t] * X[n,t,d]^2
    C2[c, j] = sum_{p,d} colsum[c, p]  * Y[j,p,d]^2
    C3[n, j] = sum_{p,d} XW[n,p,d] * Y[j,p,d],  XW = pi_c.T @ X (warp)

Sharding: 4x2 grid. Core (r, cj) owns the samples of classes {2r, 2r+1}
(zero-padded to cap1 rows per class, NL = 2*cap1) and the j-half
[512*cj, 512*(cj+1)).  One SPMD program for all 8 cores; per-core class
structure enters only through data (pis/aux tensors), all inputs are
host-quantized to fp8e4 (values << 240, encodings match e4m3).

All contractions run on the PE at fp8 DoubleRow rate (0.5 cyc/row):
colsum/rowsum as ones-contractions of pi; warp XW = pi.T @ X; C3 =
xwt.T @ ytl over k=(pc*8+d, p); C2 folds colsum*sum_d Y^2 into one DR
chain against squared Y with the colsum replicated over d; C1 likewise
contracts rowsum over (t,tc,d) k-tiles directly against squared X; the
per-sample C1 and per-class C2 rows are added by two fp16 rank-2
augmentation matmuls per 128-row block into the same psum group.
Squares are fp8 elementwise work split across ACT/DVE/Pool.  Details:
  - piS+piT merged into one input DMA; ytl chunks go via Pool SWDGE so
    the HWDGE pipeline only carries pis/xt2/aux (earlier last-input).
  - rowsum computed right after colsum (pi arrives first).
  - aug folded to ONE rank-4 fp16 matmul per n-block using a host-
    shipped aux tensor [4, NL+512]: rows0-1 ind, rows2-3 zeros (augB
    DMA'd in at runtime) | cols NL:: rows0-1 c2row (evac'd), rows2-3
    ones (shipped).
  - XW/out evacs spread across ACT/DVE/Pool.
  - 3 per-block output DMAs, pipelined.
"""

import sys

sys.path.insert(0, "/opt/trn_rl_repo")

import numpy as np

N, NY, T, TP, D, C = 1024, 1024, 256, 256, 8, 8
NCORES = 8
NYL = 512  # j columns per core

_cache = {}

POOL_PSUM = False  # Pool engine reads PSUM for some evacs


def _build(cap1):
    import concourse.bacc as bacc
    import concourse.mybir as mybir
    import concourse.tile as tile

    f8 = mybir.dt.float8e4
    bf = mybir.dt.bfloat16
    f16 = mybir.dt.float16
    f32 = mybir.dt.float32
    DR = mybir.MatmulPerfMode.DoubleRow
    NL = 2 * cap1
    SL = cap1 * 8  # slot boundary in (n,d) columns

    nc = bacc.Bacc("TRN2", target_bir_lowering=False, debug=False, num_devices=NCORES)

    pis_d = nc.dram_tensor("pis", [128, 2048], f8, kind="ExternalInput")
    xt2_d = nc.dram_tensor("xt2", [128, 2 * NL * 8], f8, kind="ExternalInput")
    ytl_d = nc.dram_tensor("ytl", [128, 16 * NYL], f8, kind="ExternalInput")
    aux_d = nc.dram_tensor("aux", [4, NL + 2 * NYL], f16, kind="ExternalInput")
    out_d = nc.dram_tensor("out", [NL, NYL], bf, kind="ExternalOutput")

    NB = [(i, min(128, NL - i)) for i in range(0, NL, 128)]

    # engine split of each 2048-col ysq chunk: (ACT, DVE, rest=Pool)
    YA, YD = 850, 800
    # engine split of each xsq chunk (fractions of SL per tc)
    XA, XD = 0.40, 0.40

    with tile.TileContext(nc) as tc:
        with (
            tc.tile_pool(name="io", bufs=1) as pio,
            tc.tile_pool(name="work", bufs=1) as pw,
            tc.tile_pool(name="small", bufs=1) as psm,
            tc.tile_pool(name="ps", bufs=1, space="PSUM") as pp,
        ):
            pis = pio.tile([128, 2048], f8, tag="pis")
            xt2 = pio.tile([128, 2 * NL * 8], f8, tag="xt2")
            ytl = pio.tile([128, 16 * NYL], f8, tag="ytl")
            aux = psm.tile([4, NL + 2 * NYL], f16, tag="aux")
            xt2v = xt2.rearrange("l (t nd) -> l t nd", t=2)
            xt2dv = xt2_d.rearrange("l (t nd) -> l t nd", t=2)
            ytlv = ytl.rearrange("l (kc j) -> l kc j", kc=16)

            # ---- input DMAs: pis/xt2/aux on SP HWDGE, ytl on Pool SWDGE ---
            def ydma(q, eng):
                eng.dma_start(
                    ytl[:, q * 4 * NYL : (q + 1) * 4 * NYL],
                    ytl_d[:, q * 4 * NYL : (q + 1) * 4 * NYL],
                )

            nc.sync.dma_start(pis[:], pis_d[:, :])
            ydma(0, nc.gpsimd)
            ydma(2, nc.gpsimd)
            nc.sync.dma_start(xt2v[:, :, 0:SL], xt2dv[:, :, 0:SL])
            ydma(1, nc.sync)
            nc.sync.dma_start(xt2v[:, :, SL:], xt2dv[:, :, SL:])
            ydma(3, nc.sync)
            nc.sync.dma_start(aux[:], aux_d[:, :])

            piSv = pis[:, 0:1024].rearrange("l (c t p) -> l c t p", c=2, t=2)
            piTv = pis[:, 1024:2048].rearrange("l (c pc t) -> l c pc t", c=2, pc=2)

            ones8 = psm.tile([128, 2], f8, tag="ones8")
            nc.vector.memset(ones8[:], 1.0)
            ones8v = ones8.rearrange("l (t o) -> l t o", o=1)
            # ACT square-table preload (1.3us) during the DMA window
            dummy8 = psm.tile([128, 2], f8, tag="dummy8")
            nc.scalar.square(dummy8[:], ones8[:])

            # ---- colsum + rowsum (PE) + evacs (DVE) -----------------------
            csrs = pp.tile([128, 16], f32, tag="pssmall", bufs=2, name="csrs")
            for c in range(2):
                for pc in range(2):
                    nc.tensor.matmul(
                        csrs[:, 2 * c + pc : 2 * c + pc + 1],
                        piSv[:, c, :, pc * 128 : (pc + 1) * 128],
                        ones8v[:],
                        start=True, stop=True, perf_mode=DR,
                        skip_group_check=True,
                    )
            for c in range(2):
                for tcc in range(2):
                    nc.tensor.matmul(
                        csrs[:, 8 + 2 * c + tcc : 8 + 2 * c + tcc + 1],
                        piTv[:, c, :, tcc * 128 : (tcc + 1) * 128],
                        ones8v[:],
                        start=True, stop=True, perf_mode=DR,
                        skip_group_check=True,
                    )
            colsR = psm.tile([128, 64], f8, tag="colsR")
            colsRv = colsR.rearrange("l (pc k c) -> l pc k c", pc=2, k=2)
            cspsv = csrs[:, 0:4].rearrange("l (c pc) -> l pc c", c=2)
            rowsS2 = psm.tile([128, 64], f8, tag="rowsS2")
            rowsS2v = rowsS2.rearrange("l (t k c) -> l t k c", t=2, k=2)
            rspsv = csrs[:, 8:12].rearrange("l (c t) -> l t c", c=2)
            for k in range(2):
                nc.vector.tensor_copy(colsRv[:, :, k, 0:2], cspsv)
                nc.vector.tensor_copy(rowsS2v[:, :, k, 0:2], rspsv)

            # ---- warp matmuls (PE), evacs deferred ------------------------
            xwt = pw.tile([128, 16 * NL], f8, tag="xwt")
            xwtv = xwt.rearrange("l (kc n) -> l kc n", kc=16)
            evacs = []
            for pc in (0, 1):
                for s in (0, 1):
                    for c0 in range(s * SL, (s + 1) * SL, 512):
                        c1 = min(c0 + 512, (s + 1) * SL)
                        w = pp.tile(
                            [128, 512], f32, tag="psxw", bufs=3,
                            name=f"xw{pc}s{s}c{c0}",
                        )
                        wv = w[:, 0 : c1 - c0]
                        nc.tensor.matmul(
                            wv,
                            piSv[:, s, :, pc * 128 : (pc + 1) * 128],
                            xt2v[:, :, c0:c1],
                            start=True, stop=True, perf_mode=DR,
                            skip_group_check=True,
                        )
                        evacs.append((wv, c0, c1, pc))

            def xw_evac(i, eng):
                wv, c0, c1, pc = evacs[i]
                dst = xwtv[:, pc * 8 : (pc + 1) * 8, c0 // 8 : c1 // 8]
                src = wv.rearrange("l (n d) -> l d n", d=8)
                if eng == "a":
                    nc.scalar.mul(dst, src, -2.0)
                elif eng == "d":
                    nc.vector.tensor_scalar_mul(dst, src, -2.0)
                else:
                    nc.gpsimd.tensor_scalar_mul(dst, src, -2.0)

            # ---- squares --------------------------------------------------
            ysq = pw.tile([128, 16 * NYL], f8, tag="ysq")
            ysqv = ysq.rearrange("l (kc j) -> l kc j", kc=16)

            def ysq_chunk(q, nq=1):
                c0 = q * 4 * NYL
                ya, yd = YA * nq, YD * nq
                nc.scalar.square(ysq[:, c0 : c0 + ya], ytl[:, c0 : c0 + ya])
                nc.vector.tensor_mul(
                    ysq[:, c0 + ya : c0 + ya + yd],
                    ytl[:, c0 + ya : c0 + ya + yd],
                    ytl[:, c0 + ya : c0 + ya + yd],
                )
                nc.gpsimd.tensor_mul(
                    ysq[:, c0 + ya + yd : c0 + nq * 4 * NYL],
                    ytl[:, c0 + ya + yd : c0 + nq * 4 * NYL],
                    ytl[:, c0 + ya + yd : c0 + nq * 4 * NYL],
                )

            xsq = pw.tile([128, 2 * NL * 8], f8, tag="xsq")
            xsqv = xsq.rearrange("l (t nd) -> l t nd", t=2)

            def xsq_chunk(s):
                h0, h1 = s * SL, (s + 1) * SL
                aa = int(SL * XA)
                dd = int(SL * XD)
                nc.scalar.square(
                    xsqv[:, :, h0 : h0 + aa], xt2v[:, :, h0 : h0 + aa]
                )
                nc.vector.tensor_mul(
                    xsqv[:, :, h0 + aa : h0 + aa + dd],
                    xt2v[:, :, h0 + aa : h0 + aa + dd],
                    xt2v[:, :, h0 + aa : h0 + aa + dd],
                )
                nc.gpsimd.tensor_mul(
                    xsqv[:, :, h0 + aa + dd : h1],
                    xt2v[:, :, h0 + aa + dd : h1],
                    xt2v[:, :, h0 + aa + dd : h1],
                )

            E = ("a", "d", "p") if POOL_PSUM else ("a", "d", "a", "d")
            ysq_chunk(0)
            xsq_chunk(0)
            xw_evac(0, E[0])
            xw_evac(1, E[1])
            xw_evac(2, E[2])
            ysq_chunk(1)
            xw_evac(3, E[3])
            xw_evac(4, E[0])
            xw_evac(5, E[1])
            xsq_chunk(1)
            xw_evac(6, E[2])
            xw_evac(7, E[3])
            xw_evac(8, E[0])
            ysq_chunk(2)
            xw_evac(9, E[1])
            xw_evac(10, E[2])
            xw_evac(11, E[3])
            ysq_chunk(3)

            # ---- C2 chain + C3 (PE) ---------------------------------------
            c2ps = pp.tile([2, NYL], f32, tag="pssmall", bufs=2, name="c2ps")

            def c2dr(pc, m, start, stop):
                kc = pc * 8 + 2 * m
                nc.tensor.matmul(
                    c2ps[:],
                    colsRv[:, pc, :, 0:2],
                    ysqv[:, kc : kc + 2, :],
                    start=start, stop=stop, perf_mode=DR,
                    skip_group_check=True,
                )

            outps = [
                pp.tile([pn, NYL], f32, tag="psout", bufs=3, name=f"outps{ib}")
                for ib, (i0, pn) in enumerate(NB)
            ]

            def c3dr(pc, m, start):
                kc = pc * 8 + 2 * m
                for ib, (i0, pn) in enumerate(NB):
                    nc.tensor.matmul(
                        outps[ib][:],
                        xwtv[:, kc : kc + 2, i0 : i0 + pn],
                        ytlv[:, kc : kc + 2, :],
                        start=start, stop=False, perf_mode=DR,
                        skip_group_check=True,
                    )

            for m in range(4):
                c2dr(0, m, m == 0, False)
            for m in range(4):
                c3dr(0, m, m == 0)
            c3dr(1, 0, False)
            c3dr(1, 1, False)
            c2dr(1, 0, False, False)
            c2dr(1, 1, False, False)

            # C1 chain (needs xsq complete + rowsS2)
            xsq4 = xsq.rearrange("l (t n d) -> l t d n", t=2, d=8)
            c1ps = pp.tile([2, NL], f32, tag="pssmall", bufs=2, name="c1ps")
            ci = 0
            for tcc in range(2):
                for m in range(4):
                    nc.tensor.matmul(
                        c1ps[:],
                        rowsS2v[:, tcc, :, 0:2],
                        xsq4[:, tcc, 2 * m : 2 * m + 2, :],
                        start=(ci == 0), stop=(ci == 7), perf_mode=DR,
                        skip_group_check=True,
                    )
                    ci += 1

            # c1c evac, augB = ind*c1c, DMA into aux rows 2-3 (off-tail)
            c1c = psm.tile([2, NL], f16, tag="c1c")
            nc.vector.tensor_copy(c1c[:], c1ps[:])
            augB = psm.tile([2, NL], f16, tag="augB")
            nc.vector.tensor_mul(augB[:], aux[0:2, 0:NL], c1c[:])

            c2dr(1, 2, False, False)
            c2dr(1, 3, False, True)
            c3dr(1, 2, False)
            c3dr(1, 3, False)

            # ---- tail: c2row evac into aux, rank-4 augs, out --------------
            nc.scalar.mul(aux[0:2, NL : NL + NYL], c2ps[:], 1.0)

            outsb = pw.tile([128, 3 * NYL], bf, tag="outsb")
            for ib, (i0, pn) in enumerate(NB):
                nc.tensor.matmul(
                    outps[ib][:],
                    augB[:, i0 : i0 + pn],
                    aux[0:2, NL + NYL : NL + 2 * NYL],
                    start=False, stop=False,
                    skip_group_check=True,
                )
            for ib, (i0, pn) in enumerate(NB):
                nc.tensor.matmul(
                    outps[ib][:],
                    aux[0:2, i0 : i0 + pn],
                    aux[0:2, NL : NL + NYL],
                    start=False, stop=True,
                    skip_group_check=True,
                )
                dst = outsb[0:pn, ib * NYL : (ib + 1) * NYL]
                if ib == 0:
                    nc.vector.tensor_copy(dst, outps[ib][:])
                elif ib == 1:
                    nc.scalar.mul(dst, outps[ib][:], 1.0)
                else:
                    if POOL_PSUM:
                        nc.gpsimd.tensor_scalar_mul(dst, outps[ib][:], 1.0)
                    else:
                        nc.scalar.mul(dst, outps[ib][:], 1.0)
                if ib == 2:
                    nc.scalar.dma_start(out_d[i0 : i0 + pn, :], dst)
                else:
                    nc.sync.dma_start(out_d[i0 : i0 + pn, :], dst)

    nc.compile()
    return nc


def kernel(X, Y, pi_dtw, classes):
    import ml_dtypes
    from concourse.bass_utils import run_bass_kernel_spmd

    f8 = ml_dtypes.float8_e4m3
    X = np.ascontiguousarray(np.asarray(X, dtype=np.float32))
    Y = np.ascontiguousarray(np.asarray(Y, dtype=np.float32))
    pi_dtw = np.ascontiguousarray(np.asarray(pi_dtw, dtype=np.float32))
    classes = np.asarray(classes).astype(np.int64)

    counts = np.bincount(classes, minlength=C)
    cap1 = int(-(-int(counts.max()) // 16) * 16)
    NL = 2 * cap1

    if cap1 not in _cache:
        _cache[cap1] = _build(cap1)
    nc = _cache[cap1]

    idx = [np.nonzero(classes == c)[0] for c in range(C)]

    ytls = []
    for cj in range(2):
        Yh = Y[cj * NYL : (cj + 1) * NYL]
        B = Yh.transpose(1, 2, 0).reshape(2, 128, 8, NYL)
        ytls.append(
            np.ascontiguousarray(
                B.transpose(1, 0, 2, 3).reshape(128, 16 * NYL)
            ).astype(f8)
        )

    in_maps = []
    for r in range(4):
        ca, cb = 2 * r, 2 * r + 1
        Xp = np.zeros((NL, T, D), dtype=np.float32)
        Xp[0 : counts[ca]] = X[idx[ca]]
        Xp[cap1 : cap1 + counts[cb]] = X[idx[cb]]
        A = Xp.transpose(1, 0, 2).reshape(2, 128, NL, D)
        xt2 = np.ascontiguousarray(
            A.transpose(1, 0, 2, 3).reshape(128, 2 * NL * D)
        ).astype(f8)

        P = pi_dtw[[ca, cb]]
        piS = P.reshape(2, 2, 128, 256).transpose(2, 0, 1, 3).reshape(128, 1024)
        PT = np.ascontiguousarray(P.transpose(0, 2, 1))
        piT = PT.reshape(2, 2, 128, 256).transpose(2, 0, 1, 3).reshape(128, 1024)
        pis = np.ascontiguousarray(
            np.concatenate([piS, piT], axis=1)
        ).astype(f8)

        aux = np.zeros((4, NL + 2 * NYL), dtype=np.float16)
        aux[0, 0:cap1] = 1.0
        aux[1, cap1:NL] = 1.0
        aux[0:2, NL + NYL :] = 1.0

        for cj in range(2):
            in_maps.append(
                {"pis": pis, "xt2": xt2, "ytl": ytls[cj], "aux": aux}
            )

    res = run_bass_kernel_spmd(nc, in_maps, core_ids=list(range(NCORES)))

    out = np.empty((N, NY), dtype=np.float32)
    jr = [np.arange(0, NYL), np.arange(NYL, NY)]
    for r in range(4):
        ca, cb = 2 * r, 2 * r + 1
        for cj in range(2):
            blk = np.asarray(res.results[2 * r + cj]["out"]).astype(np.float32)
            out[np.ix_(idx[ca], jr[cj])] = blk[0 : counts[ca]]
            out[np.ix_(idx[cb], jr[cj])] = blk[cap1 : cap1 + counts[cb]]
    return out


# revision 3
# speedup vs baseline: 1.0176x; 1.0137x over previous
"""Trainium2 Bass kernel for the CNN-MAD per-class DTW transport cost.

Math (reference):
  mat_cost[n, j] = C1[n] + C2[c_n, j] - 2*C3[n, j],  c_n = classes[n]
    C1[n]    = sum_{t,d} rowsum[c_n, t] * X[n,t,d]^2
    C2[c, j] = sum_{p,d} colsum[c, p]  * Y[j,p,d]^2
    C3[n, j] = sum_{p,d} XW[n,p,d] * Y[j,p,d],  XW = pi_c.T @ X (warp)

Sharding: 4x2 grid. Core (r, cj) owns the samples of classes {2r, 2r+1}
(zero-padded to cap1 rows per class, NL = 2*cap1) and the j-half
[512*cj, 512*(cj+1)).  One SPMD program for all 8 cores; per-core class
structure enters only through data (pis/aux tensors), all inputs are
host-quantized to fp8e4 (values << 240, encodings match e4m3).

All contractions run on the PE at fp8 DoubleRow rate (0.5 cyc/row):
colsum/rowsum as ones-contractions of pi; warp XW = pi.T @ X; C3 =
xwt.T @ ytl over k=(pc*8+d, p); C2 folds colsum*sum_d Y^2 into one DR
chain against squared Y with the colsum replicated over d; C1 likewise
contracts rowsum over (t,tc,d) k-tiles directly against squared X; the
per-sample C1 and per-class C2 rows are added by two fp16 rank-2
augmentation matmuls per 128-row block into the same psum group.
Squares are fp8 elementwise work split across ACT/DVE/Pool.  Details:
  - piS+piT merged into one input DMA; ytl chunks go via Pool SWDGE so
    the HWDGE pipeline only carries pis/xt2/aux (earlier last-input).
  - rowsum computed right after colsum (pi arrives first).
  - aug folded to ONE rank-4 fp16 matmul per n-block using a host-
    shipped aux tensor [4, NL+512]: rows0-1 ind, rows2-3 zeros (augB
    DMA'd in at runtime) | cols NL:: rows0-1 c2row (evac'd), rows2-3
    ones (shipped).
  - XW/out evacs spread across ACT/DVE/Pool.
  - 3 per-block output DMAs, pipelined.
"""

import sys

sys.path.insert(0, "/opt/trn_rl_repo")

import numpy as np

N, NY, T, TP, D, C = 1024, 1024, 256, 256, 8, 8
NCORES = 8
NYL = 512  # j columns per core

_cache = {}

POOL_PSUM = False  # Pool engine reads PSUM for some evacs


def _build(cap1):
    import concourse.bacc as bacc
    import concourse.mybir as mybir
    import concourse.tile as tile

    f8 = mybir.dt.float8e4
    bf = mybir.dt.bfloat16
    f16 = mybir.dt.float16
    f32 = mybir.dt.float32
    DR = mybir.MatmulPerfMode.DoubleRow
    NL = 2 * cap1
    SL = cap1 * 8  # slot boundary in (n,d) columns

    nc = bacc.Bacc("TRN2", target_bir_lowering=False, debug=False, num_devices=NCORES)

    pis_d = nc.dram_tensor("pis", [128, 2048], f8, kind="ExternalInput")
    xt2_d = nc.dram_tensor("xt2", [128, 2 * NL * 8], f8, kind="ExternalInput")
    ytl_d = nc.dram_tensor("ytl", [128, 16 * NYL], f8, kind="ExternalInput")
    aux_d = nc.dram_tensor("aux", [4, NL + 2 * NYL], f16, kind="ExternalInput")
    out_d = nc.dram_tensor("out", [NL, NYL], bf, kind="ExternalOutput")

    NB = [(i, min(128, NL - i)) for i in range(0, NL, 128)]

    # engine split of each 2048-col ysq chunk: (ACT, DVE, rest=Pool)
    YA, YD = 850, 800
    # engine split of each xsq chunk (fractions of SL per tc)
    XA, XD = 0.40, 0.40

    with tile.TileContext(nc) as tc:
        with (
            tc.tile_pool(name="io", bufs=1) as pio,
            tc.tile_pool(name="work", bufs=1) as pw,
            tc.tile_pool(name="small", bufs=1) as psm,
            tc.tile_pool(name="ps", bufs=1, space="PSUM") as pp,
        ):
            pis = pio.tile([128, 2048], f8, tag="pis")
            xt2 = pio.tile([128, 2 * NL * 8], f8, tag="xt2")
            ytl = pio.tile([128, 16 * NYL], f8, tag="ytl")
            aux = psm.tile([4, NL + 2 * NYL], f16, tag="aux")
            xt2v = xt2.rearrange("l (t nd) -> l t nd", t=2)
            xt2dv = xt2_d.rearrange("l (t nd) -> l t nd", t=2)
            ytlv = ytl.rearrange("l (kc j) -> l kc j", kc=16)

            # ---- input DMAs: pis/xt2/aux on SP HWDGE, ytl on Pool SWDGE ---
            def ydma(q, eng):
                eng.dma_start(
                    ytl[:, q * 4 * NYL : (q + 1) * 4 * NYL],
                    ytl_d[:, q * 4 * NYL : (q + 1) * 4 * NYL],
                )

            nc.sync.dma_start(pis[:], pis_d[:, :])
            ydma(0, nc.gpsimd)
            ydma(2, nc.gpsimd)
            nc.sync.dma_start(xt2v[:, :, 0:SL], xt2dv[:, :, 0:SL])
            ydma(1, nc.sync)
            nc.sync.dma_start(xt2v[:, :, SL:], xt2dv[:, :, SL:])
            ydma(3, nc.sync)
            nc.sync.dma_start(aux[:], aux_d[:, :])

            piSv = pis[:, 0:1024].rearrange("l (c t p) -> l c t p", c=2, t=2)
            piTv = pis[:, 1024:2048].rearrange("l (c pc t) -> l c pc t", c=2, pc=2)

            ones8 = psm.tile([128, 2], f8, tag="ones8")
            nc.vector.memset(ones8[:], 1.0)
            ones8v = ones8.rearrange("l (t o) -> l t o", o=1)
            # ACT square-table preload (1.3us) during the DMA window
            dummy8 = psm.tile([128, 2], f8, tag="dummy8")
            nc.scalar.square(dummy8[:], ones8[:])

            # ---- colsum + rowsum (PE) + evacs (DVE) -----------------------
            csrs = pp.tile([128, 16], f32, tag="pssmall", bufs=2, name="csrs")
            for c in range(2):
                for pc in range(2):
                    nc.tensor.matmul(
                        csrs[:, 2 * c + pc : 2 * c + pc + 1],
                        piSv[:, c, :, pc * 128 : (pc + 1) * 128],
                        ones8v[:],
                        start=True, stop=True, perf_mode=DR,
                        skip_group_check=True,
                    )
            for c in range(2):
                for tcc in range(2):
                    nc.tensor.matmul(
                        csrs[:, 8 + 2 * c + tcc : 8 + 2 * c + tcc + 1],
                        piTv[:, c, :, tcc * 128 : (tcc + 1) * 128],
                        ones8v[:],
                        start=True, stop=True, perf_mode=DR,
                        skip_group_check=True,
                    )
            colsR = psm.tile([128, 64], f8, tag="colsR")
            colsRv = colsR.rearrange("l (pc k c) -> l pc k c", pc=2, k=2)
            cspsv = csrs[:, 0:4].rearrange("l (c pc) -> l pc c", c=2)
            rowsS2 = psm.tile([128, 64], f8, tag="rowsS2")
            rowsS2v = rowsS2.rearrange("l (t k c) -> l t k c", t=2, k=2)
            rspsv = csrs[:, 8:12].rearrange("l (c t) -> l t c", c=2)
            for k in range(2):
                nc.vector.tensor_copy(colsRv[:, :, k, 0:2], cspsv)
                nc.vector.tensor_copy(rowsS2v[:, :, k, 0:2], rspsv)

            # ---- warp matmuls (PE), evacs deferred ------------------------
            xwt = pw.tile([128, 16 * NL], f8, tag="xwt")
            xwtv = xwt.rearrange("l (kc n) -> l kc n", kc=16)
            evacs = []
            for pc in (0, 1):
                for s in (0, 1):
                    for c0 in range(s * SL, (s + 1) * SL, 512):
                        c1 = min(c0 + 512, (s + 1) * SL)
                        w = pp.tile(
                            [128, 512], f32, tag="psxw", bufs=3,
                            name=f"xw{pc}s{s}c{c0}",
                        )
                        wv = w[:, 0 : c1 - c0]
                        nc.tensor.matmul(
                            wv,
                            piSv[:, s, :, pc * 128 : (pc + 1) * 128],
                            xt2v[:, :, c0:c1],
                            start=True, stop=True, perf_mode=DR,
                            skip_group_check=True,
                        )
                        evacs.append((wv, c0, c1, pc))

            def xw_evac(i, eng):
                wv, c0, c1, pc = evacs[i]
                dst = xwtv[:, pc * 8 : (pc + 1) * 8, c0 // 8 : c1 // 8]
                src = wv.rearrange("l (n d) -> l d n", d=8)
                if eng == "a":
                    nc.scalar.mul(dst, src, -2.0)
                elif eng == "d":
                    nc.vector.tensor_scalar_mul(dst, src, -2.0)
                else:
                    nc.gpsimd.tensor_scalar_mul(dst, src, -2.0)

            # ---- squares --------------------------------------------------
            ysq = pw.tile([128, 16 * NYL], f8, tag="ysq")
            ysqv = ysq.rearrange("l (kc j) -> l kc j", kc=16)

            def ysq_chunk(q, nq=1):
                c0 = q * 4 * NYL
                ya, yd = YA * nq, YD * nq
                nc.scalar.square(ysq[:, c0 : c0 + ya], ytl[:, c0 : c0 + ya])
                nc.vector.tensor_mul(
                    ysq[:, c0 + ya : c0 + ya + yd],
                    ytl[:, c0 + ya : c0 + ya + yd],
                    ytl[:, c0 + ya : c0 + ya + yd],
                )
                nc.gpsimd.tensor_mul(
                    ysq[:, c0 + ya + yd : c0 + nq * 4 * NYL],
                    ytl[:, c0 + ya + yd : c0 + nq * 4 * NYL],
                    ytl[:, c0 + ya + yd : c0 + nq * 4 * NYL],
                )

            xsq = pw.tile([128, 2 * NL * 8], f8, tag="xsq")
            xsqv = xsq.rearrange("l (t nd) -> l t nd", t=2)

            def xsq_chunk(s):
                h0, h1 = s * SL, (s + 1) * SL
                aa = int(SL * XA)
                dd = int(SL * XD)
                nc.scalar.square(
                    xsqv[:, :, h0 : h0 + aa], xt2v[:, :, h0 : h0 + aa]
                )
                nc.vector.tensor_mul(
                    xsqv[:, :, h0 + aa : h0 + aa + dd],
                    xt2v[:, :, h0 + aa : h0 + aa + dd],
                    xt2v[:, :, h0 + aa : h0 + aa + dd],
                )
                nc.gpsimd.tensor_mul(
                    xsqv[:, :, h0 + aa + dd : h1],
                    xt2v[:, :, h0 + aa + dd : h1],
                    xt2v[:, :, h0 + aa + dd : h1],
                )

            E = ("a", "d", "p") if POOL_PSUM else ("a", "d", "a", "d")
            ysq_chunk(0)
            xsq_chunk(0)
            xw_evac(0, E[0])
            xw_evac(1, E[1])
            xw_evac(2, E[2])
            ysq_chunk(1)
            xw_evac(3, E[3])
            xw_evac(4, E[0])
            xw_evac(5, E[1])
            xsq_chunk(1)
            xw_evac(6, E[2])
            xw_evac(7, E[3])
            xw_evac(8, E[0])
            ysq_chunk(2)
            xw_evac(9, E[1])
            xw_evac(10, E[2])
            xw_evac(11, E[3])
            ysq_chunk(3)

            # ---- C2 chain + C3 (PE) ---------------------------------------
            c2ps = pp.tile([2, NYL], f32, tag="pssmall", bufs=2, name="c2ps")

            def c2dr(pc, m, start, stop):
                kc = pc * 8 + 2 * m
                nc.tensor.matmul(
                    c2ps[:],
                    colsRv[:, pc, :, 0:2],
                    ysqv[:, kc : kc + 2, :],
                    start=start, stop=stop, perf_mode=DR,
                    skip_group_check=True,
                )

            outps = [
                pp.tile([pn, NYL], f32, tag="psout", bufs=3, name=f"outps{ib}")
                for ib, (i0, pn) in enumerate(NB)
            ]

            def c3dr(pc, m, start):
                kc = pc * 8 + 2 * m
                for ib, (i0, pn) in enumerate(NB):
                    nc.tensor.matmul(
                        outps[ib][:],
                        xwtv[:, kc : kc + 2, i0 : i0 + pn],
                        ytlv[:, kc : kc + 2, :],
                        start=start, stop=False, perf_mode=DR,
                        skip_group_check=True,
                    )

            for m in range(4):
                c2dr(0, m, m == 0, False)
            for m in range(4):
                c3dr(0, m, m == 0)
            c3dr(1, 0, False)
            c3dr(1, 1, False)
            c2dr(1, 0, False, False)
            c2dr(1, 1, False, False)

            # C1 chain (needs xsq complete + rowsS2)
            xsq4 = xsq.rearrange("l (t n d) -> l t d n", t=2, d=8)
            c1ps = pp.tile([2, NL], f32, tag="pssmall", bufs=2, name="c1ps")
            ci = 0
            for tcc in range(2):
                for m in range(4):
                    nc.tensor.matmul(
                        c1ps[:],
                        rowsS2v[:, tcc, :, 0:2],
                        xsq4[:, tcc, 2 * m : 2 * m + 2, :],
                        start=(ci == 0), stop=(ci == 7), perf_mode=DR,
                        skip_group_check=True,
                    )
                    ci += 1

            # c1c evac, augB = ind*c1c, DMA into aux rows 2-3 (off-tail)
            c1c = psm.tile([2, NL], f16, tag="c1c")
            nc.vector.tensor_copy(c1c[:], c1ps[:])
            augB = psm.tile([2, NL], f16, tag="augB")
            nc.vector.tensor_mul(augB[:], aux[0:2, 0:NL], c1c[:])

            c2dr(1, 2, False, False)
            c2dr(1, 3, False, True)
            c3dr(1, 2, False)
            c3dr(1, 3, False)

            # ---- tail: c2row evac into aux, rank-4 augs, out --------------
            nc.scalar.mul(aux[0:2, NL : NL + NYL], c2ps[:], 1.0)

            outsb = pw.tile([128, 3 * NYL], bf, tag="outsb")
            for ib, (i0, pn) in enumerate(NB):
                nc.tensor.matmul(
                    outps[ib][:],
                    augB[:, i0 : i0 + pn],
                    aux[0:2, NL + NYL : NL + 2 * NYL],
                    start=False, stop=False,
                    skip_group_check=True,
                )
            for ib, (i0, pn) in enumerate(NB):
                nc.tensor.matmul(
                    outps[ib][:],
                    aux[0:2, i0 : i0 + pn],
                    aux[0:2, NL : NL + NYL],
                    start=False, stop=True,
                    skip_group_check=True,
                )
                dst = outsb[0:pn, ib * NYL : (ib + 1) * NYL]
                if ib == 0:
                    nc.vector.tensor_copy(dst, outps[ib][:])
                elif ib == 1:
                    nc.scalar.mul(dst, outps[ib][:], 1.0)
                else:
                    nc.vector.tensor_copy(dst, outps[ib][:])
                (nc.scalar if ib == 0 else nc.sync).dma_start(
                    out_d[i0 : i0 + pn, :], dst
                )

    nc.compile()
    return nc


def kernel(X, Y, pi_dtw, classes):
    import ml_dtypes
    from concourse.bass_utils import run_bass_kernel_spmd

    f8 = ml_dtypes.float8_e4m3
    X = np.ascontiguousarray(np.asarray(X, dtype=np.float32))
    Y = np.ascontiguousarray(np.asarray(Y, dtype=np.float32))
    pi_dtw = np.ascontiguousarray(np.asarray(pi_dtw, dtype=np.float32))
    classes = np.asarray(classes).astype(np.int64)

    counts = np.bincount(classes, minlength=C)
    cap1 = int(-(-int(counts.max()) // 16) * 16)
    NL = 2 * cap1

    if cap1 not in _cache:
        _cache[cap1] = _build(cap1)
    nc = _cache[cap1]

    idx = [np.nonzero(classes == c)[0] for c in range(C)]

    ytls = []
    for cj in range(2):
        Yh = Y[cj * NYL : (cj + 1) * NYL]
        B = Yh.transpose(1, 2, 0).reshape(2, 128, 8, NYL)
        ytls.append(
            np.ascontiguousarray(
                B.transpose(1, 0, 2, 3).reshape(128, 16 * NYL)
            ).astype(f8)
        )

    in_maps = []
    for r in range(4):
        ca, cb = 2 * r, 2 * r + 1
        Xp = np.zeros((NL, T, D), dtype=np.float32)
        Xp[0 : counts[ca]] = X[idx[ca]]
        Xp[cap1 : cap1 + counts[cb]] = X[idx[cb]]
        A = Xp.transpose(1, 0, 2).reshape(2, 128, NL, D)
        xt2 = np.ascontiguousarray(
            A.transpose(1, 0, 2, 3).reshape(128, 2 * NL * D)
        ).astype(f8)

        P = pi_dtw[[ca, cb]]
        piS = P.reshape(2, 2, 128, 256).transpose(2, 0, 1, 3).reshape(128, 1024)
        PT = np.ascontiguousarray(P.transpose(0, 2, 1))
        piT = PT.reshape(2, 2, 128, 256).transpose(2, 0, 1, 3).reshape(128, 1024)
        pis = np.ascontiguousarray(
            np.concatenate([piS, piT], axis=1)
        ).astype(f8)

        aux = np.zeros((4, NL + 2 * NYL), dtype=np.float16)
        aux[0, 0:cap1] = 1.0
        aux[1, cap1:NL] = 1.0
        aux[0:2, NL + NYL :] = 1.0

        for cj in range(2):
            in_maps.append(
                {"pis": pis, "xt2": xt2, "ytl": ytls[cj], "aux": aux}
            )

    res = run_bass_kernel_spmd(nc, in_maps, core_ids=list(range(NCORES)))

    out = np.empty((N, NY), dtype=np.float32)
    jr = [np.arange(0, NYL), np.arange(NYL, NY)]
    for r in range(4):
        ca, cb = 2 * r, 2 * r + 1
        for cj in range(2):
            blk = np.asarray(res.results[2 * r + cj]["out"]).astype(np.float32)
            out[np.ix_(idx[ca], jr[cj])] = blk[0 : counts[ca]]
            out[np.ix_(idx[cb], jr[cj])] = blk[cap1 : cap1 + counts[cb]]
    return out
